# revision 1
# baseline (speedup 1.0000x reference)
"""Longformer sliding-window self-attention on 8 Trainium2 NeuronCores.

Problem: hidden_states [2, 4096, 1024], 16 heads x 64 dim, window w=256.
  q = (X@Wq + bq)/sqrt(64); k = X@Wk + bk; v = X@Wv + bv
  Block-banded attention: query block n (256 queries) attends key blocks
  n-1, n, n+1 with band |ky - qx - w| <= w plus sequence bounds.

Sharding: head-parallel. Each of the 8 cores computes a 128-column slice of
the QKV projection output (= 2 heads) for the full batch/sequence, runs the
banded attention for its 2 heads, and writes out [2, 4096, 128]. The host
concatenates slices along the embedding axis.

Device-side layout (all matmuls fp32r, 1 PE cycle/row at N>=256):
  - Host passes X^T [1024, 8192] so the projections need no on-device
    transpose of X:  Q^T/K^T/V^T [128 cols, tokens] = W_slice.T @ X^T.
  - Scores are computed transposed, S^T [key, query], per 128-key chunk:
    S^T_c = K^T_chunk.T @ Q^T -> [128, 256].  Softmax normalization sums
    (over keys = partitions) come from a ones-column appended to V, so the
    PV matmul emits both attn^T and the denominator Z; no partition-dim
    reduction is ever needed.
  - Band masks are added into the score PSUM with an identity-weight matmul
    before exp (exp of -1.25e8 == 0 exactly in fp32).
  - V is produced as V^T then PE-transposed into natural [key, dim] chunks
    (the PV stationary operand needs [key, dim]).
  - attn^T [65, 256] PSUM is PE-transposed back to [queries, 65]; the last
    column holds Z, so a reciprocal + per-partition scale finish softmax.

Sequence bounds: key chunks outside [0, S) are skipped (first/last block
contract over 4 chunks instead of 6).
"""

import numpy as np

import concourse.bass as bass
import concourse.mybir as mybir
import concourse.tile as tile
from concourse import library_config
from concourse.vector_clock import ScopedClock
from concourse.bass_utils import run_bass_kernel_spmd
from contextlib import ExitStack

# Problem shape (hardcoded per the harness contract).
B, S, E = 2, 4096, 1024
H, D, W = 16, 64, 256
NB = S // W          # 16 query blocks per sequence
NCORE = 8
HL = H // NCORE      # 2 heads per core
C = E // NCORE       # 128 projection output columns per core
TC = 512             # projection token-chunk (N of the projection matmuls)
NT = B * S // TC     # 16 projection chunks
KCH = E // 128       # 8 contraction chunks of the projection
SP = S + 2 * W       # padded key extent per sequence (offset +W)
NCH = SP // 128      # 36 key chunks per sequence in padded coords
MASKVAL = -1e9

f32 = mybir.dt.float32
f32r = mybir.dt.float32r
AF = mybir.ActivationFunctionType


class _TileContext(tile.TileContext):
    """TileContext whose exit drain splits semaphore waits.

    The walrus build in this container rejects >1 sync wait on one
    instruction ("Too many sync wait commands"), while Tile's exit drain
    accumulates one wait per outstanding semaphore.  Carry each wait on its
    own drain instruction instead.
    """

    MAX_WAITS = 1

    def _drain_and_barrier(self, tick_clock, wait_clock):
        drain_inst = self.nc.sync.drain()
        wait_clock.add_sem_waits(
            drain_inst.ins, ScopedClock({None: tick_clock.global_clock})
        )
        si = drain_inst.ins.sync_info
        waits = list(si.on_wait or []) if si is not None else []
        if len(waits) > self.MAX_WAITS:
            si.on_wait = waits[: self.MAX_WAITS]
            rest = waits[self.MAX_WAITS :]
            while rest:
                d2 = self.nc.sync.drain()
                si2 = d2.ins.sync_info
                if si2 is None:
                    si2 = mybir.SyncInfo(on_wait=[], on_update=[])
                    d2.ins.sync_info = si2
                si2.on_wait = rest[: self.MAX_WAITS]
                rest = rest[self.MAX_WAITS :]
        self.nc.all_engine_barrier()
        assert self.sems is not None
        popped = self.nc._tile_sem_poison_stack.pop()
        assert popped is self._sem_poison
        self.nc.clear_and_free_semaphores(list(self.sems.allocated().values()))
        self.nc.all_engine_barrier()


def _split_sync_waits(nc, limit=1):
    """Move excess per-instruction sem waits onto same-engine NoOp carriers.

    An engine executes its instruction stream in order, so a wait hoisted
    onto a NoOp immediately before the instruction blocks the engine at the
    same program point.
    """
    n_new = 0
    for fn in nc.m.functions:
        for bb in fn.blocks:
            out = []
            for inst in bb.instructions:
                si = getattr(inst, "sync_info", None)
                waits = list(si.on_wait) if si is not None and si.on_wait else []
                if len(waits) > limit:
                    extra = waits[: len(waits) - limit]
                    si.on_wait = waits[len(waits) - limit :]
                    while extra:
                        chunk = extra[:limit]
                        extra = extra[limit:]
                        nop = mybir.InstNoOp(
                            name=f"waitsplit-{nc.next_id()}", ins=[], outs=[]
                        )
                        nop.engine = inst.engine
                        nop.sync_info = mybir.SyncInfo(on_wait=chunk, on_update=[])
                        out.append(nop)
                        n_new += 1
                out.append(inst)
            bb.instructions = out
    return n_new


def _make_pools(tc, ctx):
    """All pools up-front (flat; loopable).  PSUM: big=4 + small=4 banks."""
    return {
        "sing": ctx.enter_context(tc.tile_pool(name="sing", bufs=1)),
        "stores": ctx.enter_context(tc.tile_pool(name="stores", bufs=1)),
        "xpool": ctx.enter_context(tc.tile_pool(name="xpool", bufs=2)),
        "vtp": ctx.enter_context(tc.tile_pool(name="vtp", bufs=2)),
        "spool": ctx.enter_context(tc.tile_pool(name="spool", bufs=3)),
        "fpool": ctx.enter_context(tc.tile_pool(name="fpool", bufs=3)),
        "psA": ctx.enter_context(tc.tile_pool(name="psA", bufs=2, space="PSUM")),
        "psB": ctx.enter_context(tc.tile_pool(name="psB", bufs=2, space="PSUM")),
    }


def _setup(nc, tc, aps, P):
    """Constants + persistent stores (emitted once, outside any loop)."""
    sing = P["sing"]
    stores = P["stores"]
    cst = {}
    cst["id_f"] = sing.tile([128, 128], f32, name="id_f")
    nc.sync.dma_start(cst["id_f"], aps["idn"])
    # packed additive band masks, added into the score PSUM with one N=512
    # identity-matmul each: variant 0 = [m0|m1], variant 1 = [m4|m5]
    cst["msk"] = sing.tile([128, 2, 2 * W], f32r, name="msk")
    nc.sync.dma_start(cst["msk"], aps["msk"].rearrange("m p x -> p m x").bitcast(f32r))
    cst["id_r"] = sing.tile([128, 128], f32r, name="id_r")
    nc.sync.dma_start(cst["id_r"], aps["idn"].bitcast(f32r))

    cst["w"] = []
    cst["b"] = []
    for nm in ("q", "k", "v"):
        w_sb = sing.tile([128, KCH, C], f32r, name=f"w{nm}_sb")
        nc.sync.dma_start(
            w_sb, aps["w" + nm].rearrange("(kc p) c -> p kc c", p=128).bitcast(f32r)
        )
        b_sb = sing.tile([128, 1], f32, name=f"b{nm}_sb")
        nc.sync.dma_start(b_sb, aps["b" + nm][:, None])
        cst["w"].append(w_sb)
        cst["b"].append(b_sb)

    cst["QT"] = stores.tile([128, B * S], f32r, name="QT")
    cst["KT"] = stores.tile([128, B * SP], f32r, name="KT")
    cst["VS"] = stores.tile([128, B, HL, NCH, D + 1], f32r, name="VS")
    # ones-column: softmax denominator accumulates through the PV matmul.
    # (memset can't write f32r in this walrus build; broadcast-DMA instead.)
    ones_bcast = bass.AP(
        tensor=aps["ones"].tensor, offset=0, ap=[[0, 128], [0, NCH - 4]]
    ).bitcast(f32r)
    for b in range(B):
        for h in range(HL):
            nc.sync.dma_start(cst["VS"][:, b, h, 2 : NCH - 2, D], ones_bcast)
    return cst


def _emit_p1(nc, tc, aps, P, cst):
    """QKV projections into the transposed stores."""
    QT, KT, VS = cst["QT"], cst["KT"], cst["VS"]
    id_f = cst["id_f"]
    xt_re = aps["xt"].rearrange("(kc p) n -> p kc n", p=128)
    for t in range(NT):
        b_t, sub_t = divmod(t, S // TC)
        toff = sub_t * TC
        xt_t = P["xpool"].tile([128, KCH, TC], f32r, tag="xt", name="xt")
        # 4 separate DMAs so the load spreads across HW-DGE queues
        for kp in range(4):
            nc.sync.dma_start(
                xt_t[:, 2 * kp : 2 * kp + 2, :],
                xt_re[:, 2 * kp : 2 * kp + 2, t * TC : (t + 1) * TC].bitcast(f32r),
            )
        for ip, nm in enumerate("qkv"):
            ps = P["psA"].tile([128, TC], f32, tag="A", name=f"ps{nm}")
            for kc in range(KCH):
                nc.tensor.matmul(
                    ps,
                    cst["w"][ip][:, kc, :],
                    xt_t[:, kc, :],
                    start=(kc == 0),
                    stop=(kc == KCH - 1),
                )
            if nm == "q":
                nc.scalar.activation(
                    QT[:, t * TC : (t + 1) * TC], ps, AF.Identity, bias=cst["b"][0]
                )
            elif nm == "k":
                off = b_t * SP + W + toff
                nc.scalar.activation(
                    KT[:, off : off + TC], ps, AF.Identity, bias=cst["b"][1]
                )
            else:
                vt = P["vtp"].tile([128, TC], f32, tag="vt", name="vt")
                nc.scalar.activation(vt, ps, AF.Identity, bias=cst["b"][2])
                ch0 = (W + toff) // 128
                for h in range(HL):
                    # 4 transposes into one PSUM tile, then a single strided
                    # copy into the 65-column V chunk layout
                    pvt = P["psB"].tile([128, 4, D], f32, tag="B", name="pvt")
                    for sub in range(TC // 128):
                        # identity slice at the same base partition as the
                        # input (matmul requires matching bases)
                        nc.tensor.transpose(
                            pvt[:, sub, :],
                            vt[h * D : (h + 1) * D, sub * 128 : (sub + 1) * 128],
                            id_f[h * D : (h + 1) * D, h * D : (h + 1) * D],
                        )
                    nc.scalar.activation(
                        VS[:, b_t, h, ch0 : ch0 + 4, 0:D], pvt, AF.Copy
                    )


def _emit_p2(nc, tc, aps, P, cst):
    """Banded attention from the stores to the output."""
    QT, KT, VS = cst["QT"], cst["KT"], cst["VS"]
    id_f, id_r, msk = cst["id_f"], cst["id_r"], cst["msk"]
    out_ap = aps["out"]
    for b in range(B):
        for h in range(HL):
            for n in range(NB):
                c_lo = 2 if n == 0 else 0
                c_hi = 4 if n == NB - 1 else 6
                # one 3-bank PSUM holds all 6 score chunks [key, query];
                # each 256-col chunk stays inside a single bank
                sps = P["psA"].tile([128, 6 * W], f32, tag="A", name="sps")
                if c_lo == 0:
                    nc.tensor.matmul(
                        sps[:, 0 : 2 * W], id_r, msk[:, 0, :], start=True, stop=False,
                        skip_group_check=True,
                    )
                if c_hi == 6:
                    nc.tensor.matmul(
                        sps[:, 4 * W : 6 * W], id_r, msk[:, 1, :], start=True,
                        stop=False, skip_group_check=True,
                    )
                for c in range(c_lo, c_hi):
                    masked = (c < 2 and c_lo == 0) or (c >= 4 and c_hi == 6)
                    koff = b * SP + n * W + c * 128
                    nc.tensor.matmul(
                        sps[:, c * W : (c + 1) * W],
                        KT[h * D : (h + 1) * D, koff : koff + 128],
                        QT[h * D : (h + 1) * D, b * S + n * W : b * S + (n + 1) * W],
                        start=not masked,
                        stop=True,
                        skip_group_check=True,
                    )
                # single exp over the whole active score strip
                ex = P["spool"].tile([128, 6 * W], f32r, tag="ex", name="ex")
                nc.scalar.activation(
                    ex[:, c_lo * W : c_hi * W],
                    sps[:, c_lo * W : c_hi * W],
                    AF.Exp,
                    scale=1.0 / np.sqrt(D),
                )
                aps_t = P["psB"].tile([D + 1, W], f32, tag="B", name="aps")
                for i, c in enumerate(range(c_lo, c_hi)):
                    nc.tensor.matmul(
                        aps_t,
                        VS[:, b, h, 2 * n + c, :],
                        ex[:, c * W : (c + 1) * W],
                        start=(i == 0),
                        stop=(c == c_hi - 1),
                    )
                # finalize: PE-transpose attn^T back to [query, dim+1]; the
                # last column holds Z, so reciprocal + per-partition scale
                # complete the softmax; output lands in natural layout
                patt = P["fpool"].tile([D + 1, W], f32, tag="patt", name="patt")
                nc.vector.tensor_copy(patt, aps_t)
                for half in range(2):
                    tp = P["psB"].tile([128, D + 1], f32, tag="B", name="tp")
                    nc.tensor.transpose(
                        tp,
                        patt[:, half * 128 : (half + 1) * 128],
                        id_f[0 : D + 1, 0 : D + 1],
                    )
                    rc = P["fpool"].tile([128, 1], f32, tag="rc", name="rc")
                    nc.vector.reciprocal(rc, tp[:, D : D + 1])
                    ao = P["fpool"].tile([128, D], f32, tag="ao", name="ao")
                    nc.vector.tensor_scalar_mul(ao, tp[:, 0:D], rc)
                    r0 = n * W + half * 128
                    nc.sync.dma_start(
                        out_ap[b, r0 : r0 + 128, h * D : (h + 1) * D], ao
                    )


def _declare_aps(nc):
    return {
        "xt": nc.dram_tensor("xt", [E, B * S], f32, kind="ExternalInput").ap(),
        "wq": nc.dram_tensor("wq", [E, C], f32, kind="ExternalInput").ap(),
        "bq": nc.dram_tensor("bq", [C], f32, kind="ExternalInput").ap(),
        "wk": nc.dram_tensor("wk", [E, C], f32, kind="ExternalInput").ap(),
        "bk": nc.dram_tensor("bk", [C], f32, kind="ExternalInput").ap(),
        "wv": nc.dram_tensor("wv", [E, C], f32, kind="ExternalInput").ap(),
        "bv": nc.dram_tensor("bv", [C], f32, kind="ExternalInput").ap(),
        "msk": nc.dram_tensor("msk", [2, 128, 2 * W], f32, kind="ExternalInput").ap(),
        "idn": nc.dram_tensor("idn", [128, 128], f32, kind="ExternalInput").ap(),
        "ones": nc.dram_tensor("ones", [1], f32, kind="ExternalInput").ap(),
        "out": nc.dram_tensor("out", [B, S, C], f32, kind="ExternalOutput").ap(),
    }


def build_program(split_waits=False, loop_n=0, p1=True, p2=True):
    """Build the SPMD Bass program (same program on all 8 cores).

    loop_n>0 wraps the body in a hardware For_i loop (timing harness).
    split_waits=True applies the 1-wait-per-instruction workaround needed by
    this container's walrus build; leave False when feeding CoreSim.
    """
    nc = bass.Bass("TRN2", target_bir_lowering=False, debug=False)
    aps = _declare_aps(nc)
    with _TileContext(nc) as tc, ExitStack() as ctx:
        P = _make_pools(tc, ctx)
        cst = _setup(nc, tc, aps, P)
        if not p1:
            # timing-only: give the stores a writer so Tile's release pass
            # doesn't see never-written tiles (values are irrelevant)
            one = bass.AP(tensor=aps["ones"].tensor, offset=0, ap=[[0, 128]])
            nc.sync.dma_start(cst["QT"][:, 0], one.bitcast(f32r))
            nc.sync.dma_start(cst["KT"][:, 0], one.bitcast(f32r))
            nc.sync.dma_start(cst["VS"][:, 0, 0, 0, 0], one.bitcast(f32r))

        def body():
            if p1:
                _emit_p1(nc, tc, aps, P, cst)
            if p2:
                _emit_p2(nc, tc, aps, P, cst)

        if loop_n > 0:
            with tc.For_i(0, loop_n, 1):
                body()
        else:
            body()
    if split_waits:
        _split_sync_waits(nc)
    return nc


def _band_masks():
    """Packed additive band masks, [2, 128, 2W]: [m0|m1] and [m4|m5].

    Score chunk c covers keys y = n*W - W + c*128 + y'; band-valid iff
    0 <= y_rel - x <= 2W, which per chunk reduces to a shifted triangle.
    """
    yy = np.arange(128, dtype=np.int64)[:, None]
    xx = np.arange(W, dtype=np.int64)[None, :]
    m0 = np.where(yy >= xx, 0.0, MASKVAL)
    m1 = np.where(yy >= xx - 128, 0.0, MASKVAL)
    m4 = np.where(yy <= xx, 0.0, MASKVAL)
    m5 = np.where(yy <= xx - 128, 0.0, MASKVAL)
    lo = np.concatenate([m0, m1], axis=1)  # [128, 512]
    hi = np.concatenate([m4, m5], axis=1)
    return np.stack([lo, hi]).astype(np.float32)


def make_in_maps(hidden_states, Wq, bq, Wk, bk, Wv, bv):
    hs = np.ascontiguousarray(np.asarray(hidden_states, dtype=np.float32))
    xt = np.ascontiguousarray(hs.reshape(B * S, E).T)
    Wq = np.asarray(Wq, dtype=np.float32)
    Wk = np.asarray(Wk, dtype=np.float32)
    Wv = np.asarray(Wv, dtype=np.float32)
    bq = np.asarray(bq, dtype=np.float32)
    bk = np.asarray(bk, dtype=np.float32)
    bv = np.asarray(bv, dtype=np.float32)
    msk = _band_masks()
    idn = np.eye(128, dtype=np.float32)
    in_maps = []
    for r in range(NCORE):
        sl = slice(r * C, (r + 1) * C)
        in_maps.append(
            {
                "xt": xt,
                "wq": np.ascontiguousarray(Wq[:, sl]),
                "bq": np.ascontiguousarray(bq[sl]),
                "wk": np.ascontiguousarray(Wk[:, sl]),
                "bk": np.ascontiguousarray(bk[sl]),
                "wv": np.ascontiguousarray(Wv[:, sl]),
                "bv": np.ascontiguousarray(bv[sl]),
                "msk": msk,
                "idn": idn,
                "ones": np.ones([1], dtype=np.float32),
            }
        )
    return in_maps


_NC_CACHE = {}


def kernel(hidden_states, Wq, bq, Wk, bk, Wv, bv):
    if "nc" not in _NC_CACHE:
        _NC_CACHE["nc"] = build_program(split_waits=True)
    nc = _NC_CACHE["nc"]
    in_maps = make_in_maps(hidden_states, Wq, bq, Wk, bk, Wv, bv)
    res = run_bass_kernel_spmd(nc, in_maps, core_ids=list(range(NCORE)))
    return assemble_out([res.results[r]["out"] for r in range(NCORE)])


def assemble_out(per_core):
    """[B, S, C] per core -> [B, S, E] full."""
    return np.ascontiguousarray(np.concatenate(per_core, axis=2)).astype(np.float32)



# revision 19
# speedup vs baseline: 1.6277x; 1.6277x over previous
"""Longformer sliding-window self-attention on 8 Trainium2 NeuronCores.

Problem: hidden_states [2, 4096, 1024], 16 heads x 64 dim, window w=256.
  q = (X@Wq + bq)/sqrt(64); k = X@Wk + bk; v = X@Wv + bv
  Banded attention: query p attends keys in [p-256, p+256] (inclusive),
  clipped to the sequence.

Sharding: 2 batches x 4 head-groups. Core r handles batch r//4 and heads
(r%4)*4..+4 (256 projection columns, stored as 2 sub-slices of 128).
Host concatenates the per-core [4096, 256] outputs.

Device-side design (per core):
  P1: X^T chunks [128, 8kc, 512] stream in; Q^T/K^T (bf16) land in
      transposed stores via f32r matmuls (N=512, 1 PE cycle/row); V^T is
      PE-transposed (bf16, 1 cycle/row) into natural [key, dim] chunks
      with a ones-column so the PV matmul also emits the softmax
      denominator Z.
  P2: per 128-query tile, 5 key chunks of 128 cover the band exactly.
      Scores S^T [key, query] via bf16 matmuls (bf16 runs 1 cycle/row at
      any moving size; f32r would be 4x slower below N=256).  Band masks
      are added by the idle Pool engine directly in PSUM; one Act exp
      yields bf16 probs; PV uses the probs as the STATIONARY operand so
      attn lands in natural [query, dim] layout with 65 moving columns
      and no transposes.  DVE divides by the Z column (stride-0
      broadcast) and batches 4 tiles per output DMA.
  P1 and P2 are interleaved (a 3-slot PSUM software pipeline, scores and
  PV emitted 2 steps apart) so Act/Pool/DVE latency hides behind PE work.
"""

import os
import numpy as np
from collections import deque

import concourse.bass as bass
import concourse.mybir as mybir
import concourse.tile as tile
from concourse.vector_clock import ScopedClock
from concourse.bass_utils import run_bass_kernel_spmd
from contextlib import ExitStack
from ml_dtypes import bfloat16

# Problem shape (hardcoded per the harness contract).
B, S, E = 2, 4096, 1024
H, D, W = 16, 64, 256
NCORE = 8
HPC = 4              # heads per core
C = HPC * D          # 256 projection output columns per core
NSUB = C // 128      # 2 store sub-slices
TC = 512             # projection token-chunk
NT = S // TC         # 8 projection chunks (single batch per core)
KCH = E // 128       # 8 contraction chunks of the projection
SP = S + 2 * W       # padded key extent (offset +W)
NCH = SP // 128      # 36 padded key chunks; valid j in [2, 34)
NTILE = S // 128     # 32 query tiles of 128
MASKVAL = -1e9
SCALE = 1.0 / np.sqrt(D)

f32 = mybir.dt.float32
f32r = mybir.dt.float32r
bf16 = mybir.dt.bfloat16
AF = mybir.ActivationFunctionType
Alu = mybir.AluOpType


class _TileContext(tile.TileContext):
    """TileContext whose exit drain splits semaphore waits.

    The walrus build in this container rejects >1 sync wait on one
    instruction ("Too many sync wait commands"), while Tile's exit drain
    accumulates one wait per outstanding semaphore.  Carry each wait on its
    own drain instruction instead.
    """

    MAX_WAITS = 1

    def _drain_and_barrier(self, tick_clock, wait_clock):
        drain_inst = self.nc.sync.drain()
        wait_clock.add_sem_waits(
            drain_inst.ins, ScopedClock({None: tick_clock.global_clock})
        )
        si = drain_inst.ins.sync_info
        waits = list(si.on_wait or []) if si is not None else []
        if len(waits) > self.MAX_WAITS:
            si.on_wait = waits[: self.MAX_WAITS]
            rest = waits[self.MAX_WAITS :]
            while rest:
                d2 = self.nc.sync.drain()
                si2 = d2.ins.sync_info
                if si2 is None:
                    si2 = mybir.SyncInfo(on_wait=[], on_update=[])
                    d2.ins.sync_info = si2
                si2.on_wait = rest[: self.MAX_WAITS]
                rest = rest[self.MAX_WAITS :]
        self.nc.all_engine_barrier()
        assert self.sems is not None
        popped = self.nc._tile_sem_poison_stack.pop()
        assert popped is self._sem_poison
        self.nc.clear_and_free_semaphores(list(self.sems.allocated().values()))
        self.nc.all_engine_barrier()


def _split_sync_waits(nc, limit=1):
    """Move excess per-instruction sem waits onto same-engine NoOp carriers."""
    n_new = 0
    for fn in nc.m.functions:
        for bb in fn.blocks:
            out = []
            for inst in bb.instructions:
                si = getattr(inst, "sync_info", None)
                waits = list(si.on_wait) if si is not None and si.on_wait else []
                if len(waits) > limit:
                    extra = waits[: len(waits) - limit]
                    si.on_wait = waits[len(waits) - limit :]
                    while extra:
                        chunk = extra[:limit]
                        extra = extra[limit:]
                        nop = mybir.InstNoOp(
                            name=f"waitsplit-{nc.next_id()}", ins=[], outs=[]
                        )
                        nop.engine = inst.engine
                        nop.sync_info = mybir.SyncInfo(on_wait=chunk, on_update=[])
                        out.append(nop)
                        n_new += 1
                out.append(inst)
            bb.instructions = out
    return n_new


def _declare_aps(nc):
    return {
        "xt": nc.dram_tensor("xt", [E, S], f32, kind="ExternalInput").ap(),
        "wq": nc.dram_tensor("wq", [E, C], f32, kind="ExternalInput").ap(),
        "bq": nc.dram_tensor("bq", [C], f32, kind="ExternalInput").ap(),
        "wk": nc.dram_tensor("wk", [E, C], f32, kind="ExternalInput").ap(),
        "bk": nc.dram_tensor("bk", [C], f32, kind="ExternalInput").ap(),
        "wv": nc.dram_tensor("wv", [E, C], f32, kind="ExternalInput").ap(),
        "bv": nc.dram_tensor("bv", [C], f32, kind="ExternalInput").ap(),
        "msk": nc.dram_tensor("msk", [2, 128, 128], bf16, kind="ExternalInput").ap(),
        "idb": nc.dram_tensor("idb", [128, 128], bf16, kind="ExternalInput").ap(),
        "idn": nc.dram_tensor("idn", [128, 128], f32, kind="ExternalInput").ap(),
        "ones": nc.dram_tensor("ones", [1], bf16, kind="ExternalInput").ap(),
        "out": nc.dram_tensor("out", [S, C], f32, kind="ExternalOutput").ap(),
    }


def _make_pools(tc, ctx):
    return {
        "sing": ctx.enter_context(tc.tile_pool(name="sing", bufs=1)),
        "stores": ctx.enter_context(tc.tile_pool(name="stores", bufs=1)),
        "xp": ctx.enter_context(tc.tile_pool(name="xp", bufs=2)),
        "vtp": ctx.enter_context(tc.tile_pool(name="vtp", bufs=2)),
        "exp": ctx.enter_context(tc.tile_pool(name="exp", bufs=3)),
        "obp": ctx.enter_context(tc.tile_pool(name="obp", bufs=8)),
        # PSUM: psP = proj ring (2x2KB) + pvt (1x2KB) = 3 banks;
        #       psS = 3 slots x 3KB (scores strip + opsum) = 4.5 banks.
        "psP": ctx.enter_context(tc.tile_pool(name="psP", bufs=2, space="PSUM")),
        "psS": ctx.enter_context(tc.tile_pool(name="psS", bufs=3, space="PSUM")),
    }


def _setup(nc, tc, aps, P):
    """Constants + persistent stores.  Weight DMAs ride the Act queue and
    small constants the Pool queue so SP starts streaming X immediately."""
    sing = P["sing"]
    stores = P["stores"]
    cst = {}
    cst["idb"] = sing.tile([128, 128], bf16, name="idb")
    nc.sync.dma_start(cst["idb"], aps["idb"])
    cst["idn"] = sing.tile([128, 128], f32, name="idn")
    nc.sync.dma_start(cst["idn"], aps["idn"])
    cst["msk"] = sing.tile([128, 2, 128], bf16, name="msk")
    nc.sync.dma_start(cst["msk"], aps["msk"].rearrange("m p x -> p m x"))

    cst["w"] = []
    cst["b"] = []
    for nm in ("q", "k", "v"):
        w_sb = sing.tile([128, KCH, C], f32r, name=f"w{nm}_sb")
        nc.sync.dma_start(
            w_sb, aps["w" + nm].rearrange("(kc p) c -> p kc c", p=128).bitcast(f32r)
        )
        b_sb = sing.tile([128, NSUB], f32, name=f"b{nm}_sb")
        nc.sync.dma_start(
            b_sb, aps["b" + nm].rearrange("(s p) -> p s", p=128)
        )
        cst["w"].append(w_sb)
        cst["b"].append(b_sb)

    cst["QT"] = stores.tile([128, NSUB, S], bf16, name="QT")
    cst["KT"] = stores.tile([128, NSUB, SP], bf16, name="KT")
    cst["VS"] = stores.tile([128, HPC, NCH, D + 1], bf16, name="VS")
    # ones-column: softmax denominator accumulates through the PV matmul.
    ones_bcast = bass.AP(
        tensor=aps["ones"].tensor, offset=0, ap=[[0, 128], [0, NCH - 4]]
    )
    for h in range(HPC):
        nc.sync.dma_start(cst["VS"][:, h, 2 : NCH - 2, D], ones_bcast)
    return cst


def _emit(nc, tc, aps, P, cst):
    STAGE = int(os.environ.get("KSTAGES", "4"))
    QT, KT, VS = cst["QT"], cst["KT"], cst["VS"]
    idb, idn, msk = cst["idb"], cst["idn"], cst["msk"]
    out_ap = aps["out"]
    xt_re = aps["xt"].rearrange("(kc p) n -> p kc n", p=128)

    P1L = os.environ.get("KP1", "full")  # q|qk|qkv|tr|full
    def emit_p1_chunk(t):
        xt_t = P["xp"].tile([128, KCH, TC], f32r, tag="xt", name="xt")
        for half in range(2):
            nc.sync.dma_start(
                xt_t[:, 4 * half : 4 * half + 4, :],
                xt_re[:, 4 * half : 4 * half + 4, t * TC : (t + 1) * TC].bitcast(
                    f32r
                ),
            )
        for s in range(NSUB):
            col = slice(s * 128, (s + 1) * 128)
            for ip, nm in enumerate("qkv"):
                ps = P["psP"].tile([128, TC], f32, tag="ps", name=f"ps{nm}")
                for kc in range(KCH):
                    nc.tensor.matmul(
                        ps,
                        cst["w"][ip][:, kc, col],
                        xt_t[:, kc, :],
                        start=(kc == 0),
                        stop=(kc == KCH - 1),
                    )
                if nm == "q":
                    nc.vector.tensor_scalar_add(
                        QT[:, s, t * TC : (t + 1) * TC],
                        ps,
                        cst["b"][0][:, s : s + 1],
                    )
                    if P1L == "q":
                        break
                elif nm == "k":
                    nc.scalar.activation(
                        KT[:, s, W + t * TC : W + (t + 1) * TC],
                        ps,
                        AF.Identity,
                        bias=cst["b"][1][:, s : s + 1],
                    )
                    if P1L == "qk":
                        break
                else:
                    vt = P["vtp"].tile([128, TC], f32, tag="vt", name="vt")
                    nc.scalar.activation(
                        vt, ps, AF.Identity, bias=cst["b"][2][:, s : s + 1]
                    )
                    if P1L == "qkv":
                        continue
                    ch0 = 2 + 4 * t
                    for hh in range(2):
                        rows = slice(hh * D, (hh + 1) * D)
                        pvt = P["psP"].tile(
                            [128, 4, D], f32, tag="ps", name="pvt"
                        )
                        for q4 in range(4):
                            nc.tensor.transpose(
                                pvt[:, q4, :],
                                vt[rows, q4 * 128 : (q4 + 1) * 128],
                                idn[rows, rows],
                            )
                        if P1L == "tr":
                            continue
                        nc.vector.tensor_copy(
                            VS[:, 2 * s + hh, ch0 : ch0 + 4, 0:D], pvt
                        )

    # --- P2 pipeline ---
    obufs = {}

    def emit_scores(h, i):
        d_lo = max(0, 2 - i)
        d_hi = min(5, 34 - i)
        sub, rows = h // 2, slice((h % 2) * D, (h % 2) * D + D)
        # slot = 2 PSUM banks exactly: scores strip cols [0:640), PV output
        # at cols [640:705).  start=True marks a whole 2KB zero-region, so
        # only the FIRST matmul per bank may carry it.
        sl = P["psS"].tile([128, 1024], f32, tag="sl", name="sl")
        for d in range(d_lo, d_hi):
            j = i + d
            masked = (d == 0 and d_lo == 0) or (d == 4 and d_hi == 5)
            nc.tensor.matmul(
                sl[:, d * 128 : (d + 1) * 128],
                KT[rows, sub, j * 128 : (j + 1) * 128],
                QT[rows, sub, i * 128 : (i + 1) * 128],
                start=(d == d_lo) or (d == 4),
                stop=not masked,
                skip_group_check=True,
            )
        # band-edge triangle masks, accumulated with bf16 identity matmuls
        if d_lo == 0:
            nc.tensor.matmul(
                sl[:, 0:128], idb, msk[:, 0], start=False, stop=True,
                skip_group_check=True,
            )
        if d_hi == 5:
            nc.tensor.matmul(
                sl[:, 512:640], idb, msk[:, 1], start=False, stop=True,
                skip_group_check=True,
            )
        if STAGE < 2:
            return (h, i, sl, None, d_lo, d_hi)
        ex = P["exp"].tile([128, 640], bf16, tag="ex", name="ex")
        nc.scalar.activation(
            ex[:, d_lo * 128 : d_hi * 128],
            sl[:, d_lo * 128 : d_hi * 128],
            AF.Exp,
            scale=SCALE,
        )
        return (h, i, sl, ex, d_lo, d_hi)

    def emit_pv(st):
        h, i, sl, ex, d_lo, d_hi = st
        if STAGE < 3:
            return
        for d in range(d_lo, d_hi):
            nc.tensor.matmul(
                sl[:, 640:705],
                ex[:, d * 128 : (d + 1) * 128],
                VS[:, h, i + d, :],
                start=(d == d_lo),
                stop=(d == d_hi - 1),
                skip_group_check=True,
            )
        if i % 4 == 0:
            obufs[h] = P["obp"].tile([128, 4, D], f32, tag="ob", name="ob")
        rc = P["obp"].tile([128, 1], f32, tag="rc", name="rc")
        nc.vector.reciprocal(rc, sl[:, 704:705])
        nc.vector.tensor_scalar_mul(obufs[h][:, i % 4, :], sl[:, 640:704], rc)
        if i % 4 == 3 and STAGE >= 4:
            i0 = i - 3
            dst = bass.AP(
                tensor=out_ap.tensor,
                offset=(i0 * 128) * C + h * D,
                ap=[[C, 128], [128 * C, 4], [1, D]],
            )
            nc.sync.dma_start(dst, obufs[h])

    # unlock(t): query tiles whose last key chunk is covered by P1 chunk t
    def t_unlock(i):
        j_max = min(i + 4, 33)
        return max(0, -(-(128 * (j_max + 1) - 256) // 512) - 1)

    unlock = {t: [] for t in range(NT)}
    for i in range(NTILE):
        unlock[t_unlock(i)].append(i)

    LAG = 2
    pend = deque()
    for t in range(NT):
        emit_p1_chunk(t)
        if STAGE < 1:
            continue
        for i in unlock[t]:
            for h in range(HPC):
                pend.append(emit_scores(h, i))
                if len(pend) > LAG:
                    emit_pv(pend.popleft())
    while pend:
        emit_pv(pend.popleft())


def build_program(split_waits=False, loop_n=0):
    nc = bass.Bass("TRN2", target_bir_lowering=False, debug=False)
    aps = _declare_aps(nc)
    with _TileContext(nc) as tc, ExitStack() as ctx:
        P = _make_pools(tc, ctx)
        cst = _setup(nc, tc, aps, P)

        def body():
            _emit(nc, tc, aps, P, cst)

        if loop_n > 0:
            with tc.For_i(0, loop_n, 1):
                body()
        else:
            body()
    if split_waits:
        _split_sync_waits(nc)
    return nc


def _band_masks():
    """Additive triangle masks [2, 128, 128] for the d=0 / d=4 key chunks.

    d=0: key-query offset y'-x' in [-127,127], valid iff y' >= x'.
    d=4: offset-512 in [-127,127], valid iff y' <= x'.
    """
    yy = np.arange(128, dtype=np.int64)[:, None]
    xx = np.arange(128, dtype=np.int64)[None, :]
    m_lo = np.where(yy >= xx, 0.0, MASKVAL)
    m_hi = np.where(yy <= xx, 0.0, MASKVAL)
    return np.stack([m_lo, m_hi]).astype(np.float32).astype(bfloat16)


def make_in_maps(hidden_states, Wq, bq, Wk, bk, Wv, bv):
    hs = np.asarray(hidden_states, dtype=np.float32)
    xts = [np.ascontiguousarray(hs[b].T) for b in range(B)]
    Wq = np.asarray(Wq, dtype=np.float32)
    Wk = np.asarray(Wk, dtype=np.float32)
    Wv = np.asarray(Wv, dtype=np.float32)
    bq = np.asarray(bq, dtype=np.float32)
    bk = np.asarray(bk, dtype=np.float32)
    bv = np.asarray(bv, dtype=np.float32)
    msk = _band_masks()
    idb = np.eye(128, dtype=np.float32).astype(bfloat16)
    idn = np.eye(128, dtype=np.float32)
    ones = np.ones([1], dtype=np.float32).astype(bfloat16)
    in_maps = []
    for r in range(NCORE):
        b_r, g = divmod(r, NCORE // B)
        sl = slice(g * C, (g + 1) * C)
        in_maps.append(
            {
                "xt": xts[b_r],
                "wq": np.ascontiguousarray(Wq[:, sl]),
                "bq": np.ascontiguousarray(bq[sl]),
                "wk": np.ascontiguousarray(Wk[:, sl]),
                "bk": np.ascontiguousarray(bk[sl]),
                "wv": np.ascontiguousarray(Wv[:, sl]),
                "bv": np.ascontiguousarray(bv[sl]),
                "msk": msk,
                "idb": idb,
                "idn": idn,
                "ones": ones,
            }
        )
    return in_maps


_NC_CACHE = {}


def kernel(hidden_states, Wq, bq, Wk, bk, Wv, bv):
    if "nc" not in _NC_CACHE:
        _NC_CACHE["nc"] = build_program(split_waits=True)
    nc = _NC_CACHE["nc"]
    in_maps = make_in_maps(hidden_states, Wq, bq, Wk, bk, Wv, bv)
    res = run_bass_kernel_spmd(nc, in_maps, core_ids=list(range(NCORE)))
    return assemble_out([res.results[r]["out"] for r in range(NCORE)])


def assemble_out(per_core):
    """8 x [S, C] -> [B, S, E]."""
    full = np.stack(
        [
            np.concatenate(per_core[b * (NCORE // B) : (b + 1) * (NCORE // B)], axis=1)
            for b in range(B)
        ]
    )
    return np.ascontiguousarray(full).astype(np.float32)


# revision 20
# speedup vs baseline: 1.9604x; 1.2044x over previous
"""Longformer sliding-window self-attention on 8 Trainium2 NeuronCores.

Problem: hidden_states [2, 4096, 1024], 16 heads x 64 dim, window w=256.
  q = (X@Wq + bq)/sqrt(64); k = X@Wk + bk; v = X@Wv + bv
  Banded attention: query p attends keys in [p-256, p+256] (inclusive),
  clipped to the sequence.

Sharding: 2 batches x 4 head-groups. Core r handles batch r//4 and heads
(r%4)*4..+4 (256 projection columns, stored as 2 sub-slices of 128).
Host concatenates the per-core [4096, 256] outputs.

Device-side design (per core):
  P1: X^T chunks [128, 8kc, 512] stream in; Q^T/K^T (bf16) land in
      transposed stores via f32r matmuls (N=512, 1 PE cycle/row); V^T is
      PE-transposed (bf16, 1 cycle/row) into natural [key, dim] chunks
      with a ones-column so the PV matmul also emits the softmax
      denominator Z.
  P2: per 128-query tile, 5 key chunks of 128 cover the band exactly.
      Scores S^T [key, query] via bf16 matmuls (bf16 runs 1 cycle/row at
      any moving size; f32r would be 4x slower below N=256).  Band masks
      are added by the idle Pool engine directly in PSUM; one Act exp
      yields bf16 probs; PV uses the probs as the STATIONARY operand so
      attn lands in natural [query, dim] layout with 65 moving columns
      and no transposes.  DVE divides by the Z column (stride-0
      broadcast) and batches 4 tiles per output DMA.
  P1 and P2 are interleaved (a 3-slot PSUM software pipeline, scores and
  PV emitted 2 steps apart) so Act/Pool/DVE latency hides behind PE work.
"""

import os
import numpy as np
from collections import deque

import concourse.bass as bass
import concourse.mybir as mybir
import concourse.tile as tile
from concourse.vector_clock import ScopedClock
from concourse.bass_utils import run_bass_kernel_spmd
from contextlib import ExitStack
from ml_dtypes import bfloat16

# Problem shape (hardcoded per the harness contract).
B, S, E = 2, 4096, 1024
H, D, W = 16, 64, 256
NCORE = 8
HPC = 4              # heads per core
C = HPC * D          # 256 projection output columns per core
NSUB = C // 128      # 2 store sub-slices
TC = 512             # projection token-chunk
NT = S // TC         # 8 projection chunks (single batch per core)
KCH = E // 128       # 8 contraction chunks of the projection
SP = S + 2 * W       # padded key extent (offset +W)
NCH = SP // 128      # 36 padded key chunks; valid j in [2, 34)
NTILE = S // 128     # 32 query tiles of 128
MASKVAL = -1e9
SCALE = 1.0 / np.sqrt(D)

f32 = mybir.dt.float32
f32r = mybir.dt.float32r
bf16 = mybir.dt.bfloat16
AF = mybir.ActivationFunctionType
Alu = mybir.AluOpType


class _TileContext(tile.TileContext):
    """TileContext whose exit drain splits semaphore waits.

    The walrus build in this container rejects >1 sync wait on one
    instruction ("Too many sync wait commands"), while Tile's exit drain
    accumulates one wait per outstanding semaphore.  Carry each wait on its
    own drain instruction instead.
    """

    MAX_WAITS = 1

    def _drain_and_barrier(self, tick_clock, wait_clock):
        drain_inst = self.nc.sync.drain()
        wait_clock.add_sem_waits(
            drain_inst.ins, ScopedClock({None: tick_clock.global_clock})
        )
        si = drain_inst.ins.sync_info
        waits = list(si.on_wait or []) if si is not None else []
        if len(waits) > self.MAX_WAITS:
            si.on_wait = waits[: self.MAX_WAITS]
            rest = waits[self.MAX_WAITS :]
            while rest:
                d2 = self.nc.sync.drain()
                si2 = d2.ins.sync_info
                if si2 is None:
                    si2 = mybir.SyncInfo(on_wait=[], on_update=[])
                    d2.ins.sync_info = si2
                si2.on_wait = rest[: self.MAX_WAITS]
                rest = rest[self.MAX_WAITS :]
        self.nc.all_engine_barrier()
        assert self.sems is not None
        popped = self.nc._tile_sem_poison_stack.pop()
        assert popped is self._sem_poison
        self.nc.clear_and_free_semaphores(list(self.sems.allocated().values()))
        self.nc.all_engine_barrier()


def _split_sync_waits(nc, limit=1):
    """Move excess per-instruction sem waits onto same-engine NoOp carriers."""
    n_new = 0
    for fn in nc.m.functions:
        for bb in fn.blocks:
            out = []
            for inst in bb.instructions:
                si = getattr(inst, "sync_info", None)
                waits = list(si.on_wait) if si is not None and si.on_wait else []
                if len(waits) > limit:
                    extra = waits[: len(waits) - limit]
                    si.on_wait = waits[len(waits) - limit :]
                    while extra:
                        chunk = extra[:limit]
                        extra = extra[limit:]
                        nop = mybir.InstNoOp(
                            name=f"waitsplit-{nc.next_id()}", ins=[], outs=[]
                        )
                        nop.engine = inst.engine
                        nop.sync_info = mybir.SyncInfo(on_wait=chunk, on_update=[])
                        out.append(nop)
                        n_new += 1
                out.append(inst)
            bb.instructions = out
    return n_new


def _declare_aps(nc):
    return {
        "xt": nc.dram_tensor("xt", [E, S], bf16, kind="ExternalInput").ap(),
        "wq": nc.dram_tensor("wq", [E, C], bf16, kind="ExternalInput").ap(),
        "bq": nc.dram_tensor("bq", [C], f32, kind="ExternalInput").ap(),
        "wk": nc.dram_tensor("wk", [E, C], bf16, kind="ExternalInput").ap(),
        "bk": nc.dram_tensor("bk", [C], f32, kind="ExternalInput").ap(),
        "wv": nc.dram_tensor("wv", [E, C], bf16, kind="ExternalInput").ap(),
        "bv": nc.dram_tensor("bv", [C], f32, kind="ExternalInput").ap(),
        "msk": nc.dram_tensor("msk", [2, 128, 128], bf16, kind="ExternalInput").ap(),
        "idb": nc.dram_tensor("idb", [128, 128], bf16, kind="ExternalInput").ap(),
        "idn": nc.dram_tensor("idn", [128, 128], f32, kind="ExternalInput").ap(),
        "ones": nc.dram_tensor("ones", [1], bf16, kind="ExternalInput").ap(),
        "out": nc.dram_tensor("out", [S, C], f32, kind="ExternalOutput").ap(),
    }


def _make_pools(tc, ctx):
    return {
        "sing": ctx.enter_context(tc.tile_pool(name="sing", bufs=1)),
        "stores": ctx.enter_context(tc.tile_pool(name="stores", bufs=1)),
        "xp": ctx.enter_context(tc.tile_pool(name="xp", bufs=2)),
        "vtp": ctx.enter_context(tc.tile_pool(name="vtp", bufs=2)),
        "exp": ctx.enter_context(tc.tile_pool(name="exp", bufs=3)),
        "obp": ctx.enter_context(tc.tile_pool(name="obp", bufs=8)),
        # PSUM: psP = proj ring (2x2KB) + pvt (1x2KB) = 3 banks;
        #       psS = 3 slots x 3KB (scores strip + opsum) = 4.5 banks.
        "psP": ctx.enter_context(tc.tile_pool(name="psP", bufs=2, space="PSUM")),
        "psS": ctx.enter_context(tc.tile_pool(name="psS", bufs=3, space="PSUM")),
    }


def _setup(nc, tc, aps, P):
    """Constants + persistent stores.  Weight DMAs ride the Act queue and
    small constants the Pool queue so SP starts streaming X immediately."""
    sing = P["sing"]
    stores = P["stores"]
    cst = {}
    cst["idb"] = sing.tile([128, 128], bf16, name="idb")
    nc.scalar.dma_start(cst["idb"], aps["idb"])
    cst["idn"] = sing.tile([128, 128], f32, name="idn")
    nc.scalar.dma_start(cst["idn"], aps["idn"])
    cst["msk"] = sing.tile([128, 2, 128], bf16, name="msk")
    nc.scalar.dma_start(cst["msk"], aps["msk"].rearrange("m p x -> p m x"))

    cst["w"] = []
    cst["b"] = []
    for nm in ("q", "k", "v"):
        w_sb = sing.tile([128, KCH, C], bf16, name=f"w{nm}_sb")
        nc.scalar.dma_start(
            w_sb, aps["w" + nm].rearrange("(kc p) c -> p kc c", p=128)
        )
        b_sb = sing.tile([128, NSUB], f32, name=f"b{nm}_sb")
        nc.scalar.dma_start(
            b_sb, aps["b" + nm].rearrange("(s p) -> p s", p=128)
        )
        cst["w"].append(w_sb)
        cst["b"].append(b_sb)

    cst["QT"] = stores.tile([128, NSUB, S], bf16, name="QT")
    cst["KT"] = stores.tile([128, NSUB, SP], bf16, name="KT")
    cst["VS"] = stores.tile([128, HPC, NCH, D + 1], bf16, name="VS")
    # ones-column: softmax denominator accumulates through the PV matmul.
    ones_bcast = bass.AP(
        tensor=aps["ones"].tensor, offset=0, ap=[[0, 128], [0, NCH - 4]]
    )
    for h in range(HPC):
        nc.scalar.dma_start(cst["VS"][:, h, 2 : NCH - 2, D], ones_bcast)
    return cst


def _emit(nc, tc, aps, P, cst):
    STAGE = int(os.environ.get("KSTAGES", "4"))
    QT, KT, VS = cst["QT"], cst["KT"], cst["VS"]
    idb, idn, msk = cst["idb"], cst["idn"], cst["msk"]
    out_ap = aps["out"]
    xt_re = aps["xt"].rearrange("(kc p) n -> p kc n", p=128)

    P1L = os.environ.get("KP1", "full")  # q|qk|qkv|tr|full
    def emit_p1_chunk(t):
        xt_t = P["xp"].tile([128, KCH, TC], bf16, tag="xt", name="xt")
        for half in range(2):
            nc.sync.dma_start(
                xt_t[:, 4 * half : 4 * half + 4, :],
                xt_re[:, 4 * half : 4 * half + 4, t * TC : (t + 1) * TC],
            )
        for s in range(NSUB):
            col = slice(s * 128, (s + 1) * 128)
            for ip, nm in enumerate("qkv"):
                ps = P["psP"].tile([128, TC], f32, tag="ps", name=f"ps{nm}")
                for kc in range(KCH):
                    nc.tensor.matmul(
                        ps,
                        cst["w"][ip][:, kc, col],
                        xt_t[:, kc, :],
                        start=(kc == 0),
                        stop=(kc == KCH - 1),
                    )
                if nm == "q":
                    nc.vector.tensor_scalar_add(
                        QT[:, s, t * TC : (t + 1) * TC],
                        ps,
                        cst["b"][0][:, s : s + 1],
                    )
                    if P1L == "q":
                        break
                elif nm == "k":
                    nc.scalar.activation(
                        KT[:, s, W + t * TC : W + (t + 1) * TC],
                        ps,
                        AF.Identity,
                        bias=cst["b"][1][:, s : s + 1],
                    )
                    if P1L == "qk":
                        break
                else:
                    vt = P["vtp"].tile([128, TC], f32, tag="vt", name="vt")
                    nc.scalar.activation(
                        vt, ps, AF.Identity, bias=cst["b"][2][:, s : s + 1]
                    )
                    if P1L == "qkv":
                        continue
                    ch0 = 2 + 4 * t
                    for hh in range(2):
                        rows = slice(hh * D, (hh + 1) * D)
                        pvt = P["psP"].tile(
                            [128, 4, D], f32, tag="ps", name="pvt"
                        )
                        for q4 in range(4):
                            nc.tensor.transpose(
                                pvt[:, q4, :],
                                vt[rows, q4 * 128 : (q4 + 1) * 128],
                                idn[rows, rows],
                            )
                        if P1L == "tr":
                            continue
                        nc.vector.tensor_copy(
                            VS[:, 2 * s + hh, ch0 : ch0 + 4, 0:D], pvt
                        )

    # --- P2 pipeline ---
    obufs = {}

    def emit_scores(h, i):
        d_lo = max(0, 2 - i)
        d_hi = min(5, 34 - i)
        sub, rows = h // 2, slice((h % 2) * D, (h % 2) * D + D)
        # slot = 2 PSUM banks exactly: scores strip cols [0:640), PV output
        # at cols [640:705).  start=True marks a whole 2KB zero-region, so
        # only the FIRST matmul per bank may carry it.
        sl = P["psS"].tile([128, 1024], f32, tag="sl", name="sl")
        for d in range(d_lo, d_hi):
            j = i + d
            masked = (d == 0 and d_lo == 0) or (d == 4 and d_hi == 5)
            nc.tensor.matmul(
                sl[:, d * 128 : (d + 1) * 128],
                KT[rows, sub, j * 128 : (j + 1) * 128],
                QT[rows, sub, i * 128 : (i + 1) * 128],
                start=(d == d_lo) or (d == 4),
                stop=not masked,
                skip_group_check=True,
            )
        # band-edge triangle masks, accumulated with bf16 identity matmuls
        if d_lo == 0:
            nc.tensor.matmul(
                sl[:, 0:128], idb, msk[:, 0], start=False, stop=True,
                skip_group_check=True,
            )
        if d_hi == 5:
            nc.tensor.matmul(
                sl[:, 512:640], idb, msk[:, 1], start=False, stop=True,
                skip_group_check=True,
            )
        if STAGE < 2:
            return (h, i, sl, None, d_lo, d_hi)
        ex = P["exp"].tile([128, 640], bf16, tag="ex", name="ex")
        nc.scalar.activation(
            ex[:, d_lo * 128 : d_hi * 128],
            sl[:, d_lo * 128 : d_hi * 128],
            AF.Exp,
            scale=SCALE,
        )
        return (h, i, sl, ex, d_lo, d_hi)

    def emit_pv(st):
        h, i, sl, ex, d_lo, d_hi = st
        if STAGE < 3:
            return
        for d in range(d_lo, d_hi):
            nc.tensor.matmul(
                sl[:, 640:705],
                ex[:, d * 128 : (d + 1) * 128],
                VS[:, h, i + d, :],
                start=(d == d_lo),
                stop=(d == d_hi - 1),
                skip_group_check=True,
            )
        if i % 4 == 0:
            obufs[h] = P["obp"].tile([128, 4, D], f32, tag="ob", name="ob")
        rc = P["obp"].tile([128, 1], f32, tag="rc", name="rc")
        nc.vector.reciprocal(rc, sl[:, 704:705])
        nc.vector.tensor_scalar_mul(obufs[h][:, i % 4, :], sl[:, 640:704], rc)
        if i % 4 == 3 and STAGE >= 4:
            i0 = i - 3
            dst = bass.AP(
                tensor=out_ap.tensor,
                offset=(i0 * 128) * C + h * D,
                ap=[[C, 128], [128 * C, 4], [1, D]],
            )
            nc.sync.dma_start(dst, obufs[h])

    # unlock(t): query tiles whose last key chunk is covered by P1 chunk t
    def t_unlock(i):
        j_max = min(i + 4, 33)
        return max(0, -(-(128 * (j_max + 1) - 256) // 512) - 1)

    unlock = {t: [] for t in range(NT)}
    for i in range(NTILE):
        unlock[t_unlock(i)].append(i)

    LAG = 2
    pend = deque()
    for t in range(NT):
        emit_p1_chunk(t)
        if STAGE < 1:
            continue
        for i in unlock[t]:
            for h in range(HPC):
                pend.append(emit_scores(h, i))
                if len(pend) > LAG:
                    emit_pv(pend.popleft())
    while pend:
        emit_pv(pend.popleft())


def build_program(split_waits=False, loop_n=0):
    nc = bass.Bass("TRN2", target_bir_lowering=False, debug=False)
    aps = _declare_aps(nc)
    with _TileContext(nc) as tc, ExitStack() as ctx:
        P = _make_pools(tc, ctx)
        cst = _setup(nc, tc, aps, P)

        def body():
            _emit(nc, tc, aps, P, cst)

        if loop_n > 0:
            with tc.For_i(0, loop_n, 1):
                body()
        else:
            body()
    if split_waits:
        _split_sync_waits(nc)
    return nc


def _band_masks():
    """Additive triangle masks [2, 128, 128] for the d=0 / d=4 key chunks.

    d=0: key-query offset y'-x' in [-127,127], valid iff y' >= x'.
    d=4: offset-512 in [-127,127], valid iff y' <= x'.
    """
    yy = np.arange(128, dtype=np.int64)[:, None]
    xx = np.arange(128, dtype=np.int64)[None, :]
    m_lo = np.where(yy >= xx, 0.0, MASKVAL)
    m_hi = np.where(yy <= xx, 0.0, MASKVAL)
    return np.stack([m_lo, m_hi]).astype(np.float32).astype(bfloat16)


def make_in_maps(hidden_states, Wq, bq, Wk, bk, Wv, bv):
    hs = np.asarray(hidden_states, dtype=np.float32)
    xts = [np.ascontiguousarray(hs[b].T).astype(bfloat16) for b in range(B)]
    Wq = np.asarray(Wq, dtype=np.float32).astype(bfloat16)
    Wk = np.asarray(Wk, dtype=np.float32).astype(bfloat16)
    Wv = np.asarray(Wv, dtype=np.float32).astype(bfloat16)
    bq = np.asarray(bq, dtype=np.float32)
    bk = np.asarray(bk, dtype=np.float32)
    bv = np.asarray(bv, dtype=np.float32)
    msk = _band_masks()
    idb = np.eye(128, dtype=np.float32).astype(bfloat16)
    idn = np.eye(128, dtype=np.float32)
    ones = np.ones([1], dtype=np.float32).astype(bfloat16)
    in_maps = []
    for r in range(NCORE):
        b_r, g = divmod(r, NCORE // B)
        sl = slice(g * C, (g + 1) * C)
        in_maps.append(
            {
                "xt": xts[b_r],
                "wq": np.ascontiguousarray(Wq[:, sl]),
                "bq": np.ascontiguousarray(bq[sl]),
                "wk": np.ascontiguousarray(Wk[:, sl]),
                "bk": np.ascontiguousarray(bk[sl]),
                "wv": np.ascontiguousarray(Wv[:, sl]),
                "bv": np.ascontiguousarray(bv[sl]),
                "msk": msk,
                "idb": idb,
                "idn": idn,
                "ones": ones,
            }
        )
    return in_maps


_NC_CACHE = {}


def kernel(hidden_states, Wq, bq, Wk, bk, Wv, bv):
    if "nc" not in _NC_CACHE:
        _NC_CACHE["nc"] = build_program(split_waits=True)
    nc = _NC_CACHE["nc"]
    in_maps = make_in_maps(hidden_states, Wq, bq, Wk, bk, Wv, bv)
    res = run_bass_kernel_spmd(nc, in_maps, core_ids=list(range(NCORE)))
    return assemble_out([res.results[r]["out"] for r in range(NCORE)])


def assemble_out(per_core):
    """8 x [S, C] -> [B, S, E]."""
    full = np.stack(
        [
            np.concatenate(per_core[b * (NCORE // B) : (b + 1) * (NCORE // B)], axis=1)
            for b in range(B)
        ]
    )
    return np.ascontiguousarray(full).astype(np.float32)


# revision 21
# speedup vs baseline: 2.0490x; 1.0452x over previous
"""Longformer sliding-window self-attention on 8 Trainium2 NeuronCores.

Problem: hidden_states [2, 4096, 1024], 16 heads x 64 dim, window w=256.
  q = (X@Wq + bq)/sqrt(64); k = X@Wk + bk; v = X@Wv + bv
  Banded attention: query p attends keys in [p-256, p+256] (inclusive),
  clipped to the sequence.

Sharding: 2 batches x 4 head-groups. Core r handles batch r//4 and heads
(r%4)*4..+4 (256 projection columns, stored as 2 sub-slices of 128).
Host concatenates the per-core [4096, 256] outputs.

Device-side design (per core):
  P1: X^T chunks [128, 8kc, 512] stream in; Q^T/K^T (bf16) land in
      transposed stores via f32r matmuls (N=512, 1 PE cycle/row); V^T is
      PE-transposed (bf16, 1 cycle/row) into natural [key, dim] chunks
      with a ones-column so the PV matmul also emits the softmax
      denominator Z.
  P2: per 128-query tile, 5 key chunks of 128 cover the band exactly.
      Scores S^T [key, query] via bf16 matmuls (bf16 runs 1 cycle/row at
      any moving size; f32r would be 4x slower below N=256).  Band masks
      are added by the idle Pool engine directly in PSUM; one Act exp
      yields bf16 probs; PV uses the probs as the STATIONARY operand so
      attn lands in natural [query, dim] layout with 65 moving columns
      and no transposes.  DVE divides by the Z column (stride-0
      broadcast) and batches 4 tiles per output DMA.
  P1 and P2 are interleaved (a 3-slot PSUM software pipeline, scores and
  PV emitted 2 steps apart) so Act/Pool/DVE latency hides behind PE work.
"""

import os
import numpy as np
from collections import deque

import concourse.bass as bass
import concourse.mybir as mybir
import concourse.tile as tile
from concourse.vector_clock import ScopedClock
from concourse.bass_utils import run_bass_kernel_spmd
from contextlib import ExitStack
from ml_dtypes import bfloat16

# Problem shape (hardcoded per the harness contract).
B, S, E = 2, 4096, 1024
H, D, W = 16, 64, 256
NCORE = 8
HPC = 4              # heads per core
C = HPC * D          # 256 projection output columns per core
NSUB = C // 128      # 2 store sub-slices
TC = 512             # projection token-chunk
NT = S // TC         # 8 projection chunks (single batch per core)
KCH = E // 128       # 8 contraction chunks of the projection
SP = S + 2 * W       # padded key extent (offset +W)
NCH = SP // 128      # 36 padded key chunks; valid j in [2, 34)
NTILE = S // 128     # 32 query tiles of 128
MASKVAL = -1e9
SCALE = 1.0 / np.sqrt(D)

f32 = mybir.dt.float32
f32r = mybir.dt.float32r
bf16 = mybir.dt.bfloat16
AF = mybir.ActivationFunctionType
Alu = mybir.AluOpType


class _TileContext(tile.TileContext):
    """TileContext whose exit drain splits semaphore waits.

    The walrus build in this container rejects >1 sync wait on one
    instruction ("Too many sync wait commands"), while Tile's exit drain
    accumulates one wait per outstanding semaphore.  Carry each wait on its
    own drain instruction instead.
    """

    MAX_WAITS = 1

    def _drain_and_barrier(self, tick_clock, wait_clock):
        drain_inst = self.nc.sync.drain()
        wait_clock.add_sem_waits(
            drain_inst.ins, ScopedClock({None: tick_clock.global_clock})
        )
        si = drain_inst.ins.sync_info
        waits = list(si.on_wait or []) if si is not None else []
        if len(waits) > self.MAX_WAITS:
            si.on_wait = waits[: self.MAX_WAITS]
            rest = waits[self.MAX_WAITS :]
            while rest:
                d2 = self.nc.sync.drain()
                si2 = d2.ins.sync_info
                if si2 is None:
                    si2 = mybir.SyncInfo(on_wait=[], on_update=[])
                    d2.ins.sync_info = si2
                si2.on_wait = rest[: self.MAX_WAITS]
                rest = rest[self.MAX_WAITS :]
        self.nc.all_engine_barrier()
        assert self.sems is not None
        popped = self.nc._tile_sem_poison_stack.pop()
        assert popped is self._sem_poison
        self.nc.clear_and_free_semaphores(list(self.sems.allocated().values()))
        self.nc.all_engine_barrier()


def _split_sync_waits(nc, limit=1):
    """Move excess per-instruction sem waits onto same-engine NoOp carriers."""
    n_new = 0
    for fn in nc.m.functions:
        for bb in fn.blocks:
            out = []
            for inst in bb.instructions:
                si = getattr(inst, "sync_info", None)
                waits = list(si.on_wait) if si is not None and si.on_wait else []
                if len(waits) > limit:
                    extra = waits[: len(waits) - limit]
                    si.on_wait = waits[len(waits) - limit :]
                    while extra:
                        chunk = extra[:limit]
                        extra = extra[limit:]
                        nop = mybir.InstNoOp(
                            name=f"waitsplit-{nc.next_id()}", ins=[], outs=[]
                        )
                        nop.engine = inst.engine
                        nop.sync_info = mybir.SyncInfo(on_wait=chunk, on_update=[])
                        out.append(nop)
                        n_new += 1
                out.append(inst)
            bb.instructions = out
    return n_new


def _declare_aps(nc):
    return {
        "xt": nc.dram_tensor("xt", [E, S], bf16, kind="ExternalInput").ap(),
        "wq": nc.dram_tensor("wq", [E, C], bf16, kind="ExternalInput").ap(),
        "bq": nc.dram_tensor("bq", [C], f32, kind="ExternalInput").ap(),
        "wk": nc.dram_tensor("wk", [E, C], bf16, kind="ExternalInput").ap(),
        "bk": nc.dram_tensor("bk", [C], f32, kind="ExternalInput").ap(),
        "wv": nc.dram_tensor("wv", [E, C], bf16, kind="ExternalInput").ap(),
        "bv": nc.dram_tensor("bv", [C], f32, kind="ExternalInput").ap(),
        "msk": nc.dram_tensor("msk", [2, 128, 128], bf16, kind="ExternalInput").ap(),
        "idb": nc.dram_tensor("idb", [128, 128], bf16, kind="ExternalInput").ap(),
        "idn": nc.dram_tensor("idn", [128, 128], f32, kind="ExternalInput").ap(),
        "ones": nc.dram_tensor("ones", [1], bf16, kind="ExternalInput").ap(),
        "out": nc.dram_tensor("out", [S, C], f32, kind="ExternalOutput").ap(),
    }


def _make_pools(tc, ctx):
    return {
        "sing": ctx.enter_context(tc.tile_pool(name="sing", bufs=1)),
        "stores": ctx.enter_context(tc.tile_pool(name="stores", bufs=1)),
        "xp": ctx.enter_context(tc.tile_pool(name="xp", bufs=2)),
        "vtp": ctx.enter_context(tc.tile_pool(name="vtp", bufs=2)),
        "exp": ctx.enter_context(tc.tile_pool(name="exp", bufs=3)),
        "obp": ctx.enter_context(tc.tile_pool(name="obp", bufs=8)),
        # PSUM: psP = proj ring (2x2KB) + pvt (1x2KB) = 3 banks;
        #       psS = 3 slots x 3KB (scores strip + opsum) = 4.5 banks.
        "psP": ctx.enter_context(tc.tile_pool(name="psP", bufs=2, space="PSUM")),
        "psS": ctx.enter_context(tc.tile_pool(name="psS", bufs=3, space="PSUM")),
    }


def _setup(nc, tc, aps, P):
    """Constants + persistent stores.  Weight DMAs ride the Act queue and
    small constants the Pool queue so SP starts streaming X immediately."""
    sing = P["sing"]
    stores = P["stores"]
    cst = {}
    cst["idb"] = sing.tile([128, 128], bf16, name="idb")
    nc.scalar.dma_start(cst["idb"], aps["idb"])
    cst["idn"] = sing.tile([128, 128], f32, name="idn")
    nc.scalar.dma_start(cst["idn"], aps["idn"])
    cst["msk"] = sing.tile([128, 2, 128], bf16, name="msk")
    nc.scalar.dma_start(cst["msk"], aps["msk"].rearrange("m p x -> p m x"))

    cst["w"] = []
    cst["b"] = []
    for nm in ("q", "k", "v"):
        w_sb = sing.tile([128, KCH, C], bf16, name=f"w{nm}_sb")
        nc.scalar.dma_start(
            w_sb, aps["w" + nm].rearrange("(kc p) c -> p kc c", p=128)
        )
        b_sb = sing.tile([128, NSUB], f32, name=f"b{nm}_sb")
        nc.scalar.dma_start(
            b_sb, aps["b" + nm].rearrange("(s p) -> p s", p=128)
        )
        cst["w"].append(w_sb)
        cst["b"].append(b_sb)

    cst["QT"] = stores.tile([128, NSUB, S], bf16, name="QT")
    cst["KT"] = stores.tile([128, NSUB, SP], bf16, name="KT")
    cst["VS"] = stores.tile([128, HPC, NCH, D + 1], bf16, name="VS")
    # ones-column: softmax denominator accumulates through the PV matmul.
    ones_bcast = bass.AP(
        tensor=aps["ones"].tensor, offset=0, ap=[[0, 128], [0, NCH - 4]]
    )
    for h in range(HPC):
        nc.scalar.dma_start(cst["VS"][:, h, 2 : NCH - 2, D], ones_bcast)
    return cst


def _emit(nc, tc, aps, P, cst):
    STAGE = int(os.environ.get("KSTAGES", "4"))
    QT, KT, VS = cst["QT"], cst["KT"], cst["VS"]
    idb, idn, msk = cst["idb"], cst["idn"], cst["msk"]
    out_ap = aps["out"]
    xt_re = aps["xt"].rearrange("(kc p) n -> p kc n", p=128)

    P1L = os.environ.get("KP1", "full")  # q|qk|qkv|tr|full
    def emit_p1_chunk(t):
        xt_t = P["xp"].tile([128, KCH, TC], bf16, tag="xt", name="xt")
        nsplit = int(os.environ.get("KXSPLIT", "2"))
        kper = KCH // nsplit
        for part in range(nsplit):
            nc.sync.dma_start(
                xt_t[:, kper * part : kper * (part + 1), :],
                xt_re[:, kper * part : kper * (part + 1), t * TC : (t + 1) * TC],
            )
        for s in range(NSUB):
            col = slice(s * 128, (s + 1) * 128)
            for ip, nm in enumerate("qkv"):
                ps = P["psP"].tile([128, TC], f32, tag="ps", name=f"ps{nm}")
                for kc in range(KCH):
                    nc.tensor.matmul(
                        ps,
                        cst["w"][ip][:, kc, col],
                        xt_t[:, kc, :],
                        start=(kc == 0),
                        stop=(kc == KCH - 1),
                    )
                if nm == "q":
                    nc.vector.tensor_scalar_add(
                        QT[:, s, t * TC : (t + 1) * TC],
                        ps,
                        cst["b"][0][:, s : s + 1],
                    )
                    if P1L == "q":
                        break
                elif nm == "k":
                    nc.scalar.activation(
                        KT[:, s, W + t * TC : W + (t + 1) * TC],
                        ps,
                        AF.Identity,
                        bias=cst["b"][1][:, s : s + 1],
                    )
                    if P1L == "qk":
                        break
                else:
                    vt = P["vtp"].tile([128, TC], f32, tag="vt", name="vt")
                    nc.scalar.activation(
                        vt, ps, AF.Identity, bias=cst["b"][2][:, s : s + 1]
                    )
                    if P1L == "qkv":
                        continue
                    ch0 = 2 + 4 * t
                    for hh in range(2):
                        rows = slice(hh * D, (hh + 1) * D)
                        pvt = P["psP"].tile(
                            [128, 4, D], f32, tag="ps", name="pvt"
                        )
                        for q4 in range(4):
                            nc.tensor.transpose(
                                pvt[:, q4, :],
                                vt[rows, q4 * 128 : (q4 + 1) * 128],
                                idn[rows, rows],
                            )
                        if P1L == "tr":
                            continue
                        nc.vector.tensor_copy(
                            VS[:, 2 * s + hh, ch0 : ch0 + 4, 0:D], pvt
                        )

    # --- P2 pipeline ---
    obufs = {}

    def emit_scores(h, i):
        d_lo = max(0, 2 - i)
        d_hi = min(5, 34 - i)
        sub, rows = h // 2, slice((h % 2) * D, (h % 2) * D + D)
        # slot = 2 PSUM banks exactly: scores strip cols [0:640), PV output
        # at cols [640:705).  start=True marks a whole 2KB zero-region, so
        # only the FIRST matmul per bank may carry it.
        sl = P["psS"].tile([128, 1024], f32, tag="sl", name="sl")
        for d in range(d_lo, d_hi):
            j = i + d
            masked = (d == 0 and d_lo == 0) or (d == 4 and d_hi == 5)
            nc.tensor.matmul(
                sl[:, d * 128 : (d + 1) * 128],
                KT[rows, sub, j * 128 : (j + 1) * 128],
                QT[rows, sub, i * 128 : (i + 1) * 128],
                start=(d == d_lo) or (d == 4),
                stop=not masked,
                skip_group_check=True,
            )
        # band-edge triangle masks, accumulated with bf16 identity matmuls
        if d_lo == 0:
            nc.tensor.matmul(
                sl[:, 0:128], idb, msk[:, 0], start=False, stop=True,
                skip_group_check=True,
            )
        if d_hi == 5:
            nc.tensor.matmul(
                sl[:, 512:640], idb, msk[:, 1], start=False, stop=True,
                skip_group_check=True,
            )
        if STAGE < 2:
            return (h, i, sl, None, d_lo, d_hi)
        ex = P["exp"].tile([128, 640], bf16, tag="ex", name="ex")
        nc.scalar.activation(
            ex[:, d_lo * 128 : d_hi * 128],
            sl[:, d_lo * 128 : d_hi * 128],
            AF.Exp,
            scale=SCALE,
        )
        return (h, i, sl, ex, d_lo, d_hi)

    def emit_pv(st):
        h, i, sl, ex, d_lo, d_hi = st
        if STAGE < 3:
            return
        for d in range(d_lo, d_hi):
            nc.tensor.matmul(
                sl[:, 640:705],
                ex[:, d * 128 : (d + 1) * 128],
                VS[:, h, i + d, :],
                start=(d == d_lo),
                stop=(d == d_hi - 1),
                skip_group_check=True,
            )
        OB = int(os.environ.get("KOBATCH", "4"))
        if i % OB == 0:
            obufs[h] = P["obp"].tile([128, OB, D], f32, tag="ob", name="ob")
        rc = P["obp"].tile([128, 1], f32, tag="rc", name="rc")
        nc.vector.reciprocal(rc, sl[:, 704:705])
        nc.vector.tensor_scalar_mul(obufs[h][:, i % OB, :], sl[:, 640:704], rc)
        if i % OB == OB - 1 and STAGE >= 4:
            i0 = i - (OB - 1)
            dst = bass.AP(
                tensor=out_ap.tensor,
                offset=(i0 * 128) * C + h * D,
                ap=[[C, 128], [128 * C, OB], [1, D]],
            )
            nc.sync.dma_start(dst, obufs[h])

    # unlock(t): query tiles whose last key chunk is covered by P1 chunk t
    def t_unlock(i):
        j_max = min(i + 4, 33)
        return max(0, -(-(128 * (j_max + 1) - 256) // 512) - 1)

    unlock = {t: [] for t in range(NT)}
    for i in range(NTILE):
        unlock[t_unlock(i)].append(i)

    LAG = 2
    pend = deque()
    for t in range(NT):
        emit_p1_chunk(t)
        if STAGE < 1:
            continue
        for i in unlock[t]:
            for h in range(HPC):
                pend.append(emit_scores(h, i))
                if len(pend) > LAG:
                    emit_pv(pend.popleft())
    while pend:
        emit_pv(pend.popleft())


def build_program(split_waits=False, loop_n=0):
    nc = bass.Bass("TRN2", target_bir_lowering=False, debug=False)
    aps = _declare_aps(nc)
    with _TileContext(nc) as tc, ExitStack() as ctx:
        P = _make_pools(tc, ctx)
        cst = _setup(nc, tc, aps, P)

        def body():
            _emit(nc, tc, aps, P, cst)

        if loop_n > 0:
            with tc.For_i(0, loop_n, 1):
                body()
        else:
            body()
    if split_waits:
        _split_sync_waits(nc)
    return nc


def _band_masks():
    """Additive triangle masks [2, 128, 128] for the d=0 / d=4 key chunks.

    d=0: key-query offset y'-x' in [-127,127], valid iff y' >= x'.
    d=4: offset-512 in [-127,127], valid iff y' <= x'.
    """
    yy = np.arange(128, dtype=np.int64)[:, None]
    xx = np.arange(128, dtype=np.int64)[None, :]
    m_lo = np.where(yy >= xx, 0.0, MASKVAL)
    m_hi = np.where(yy <= xx, 0.0, MASKVAL)
    return np.stack([m_lo, m_hi]).astype(np.float32).astype(bfloat16)


def make_in_maps(hidden_states, Wq, bq, Wk, bk, Wv, bv):
    hs = np.asarray(hidden_states, dtype=np.float32)
    xts = [np.ascontiguousarray(hs[b].T).astype(bfloat16) for b in range(B)]
    Wq = np.asarray(Wq, dtype=np.float32).astype(bfloat16)
    Wk = np.asarray(Wk, dtype=np.float32).astype(bfloat16)
    Wv = np.asarray(Wv, dtype=np.float32).astype(bfloat16)
    bq = np.asarray(bq, dtype=np.float32)
    bk = np.asarray(bk, dtype=np.float32)
    bv = np.asarray(bv, dtype=np.float32)
    msk = _band_masks()
    idb = np.eye(128, dtype=np.float32).astype(bfloat16)
    idn = np.eye(128, dtype=np.float32)
    ones = np.ones([1], dtype=np.float32).astype(bfloat16)
    in_maps = []
    for r in range(NCORE):
        b_r, g = divmod(r, NCORE // B)
        sl = slice(g * C, (g + 1) * C)
        in_maps.append(
            {
                "xt": xts[b_r],
                "wq": np.ascontiguousarray(Wq[:, sl]),
                "bq": np.ascontiguousarray(bq[sl]),
                "wk": np.ascontiguousarray(Wk[:, sl]),
                "bk": np.ascontiguousarray(bk[sl]),
                "wv": np.ascontiguousarray(Wv[:, sl]),
                "bv": np.ascontiguousarray(bv[sl]),
                "msk": msk,
                "idb": idb,
                "idn": idn,
                "ones": ones,
            }
        )
    return in_maps


_NC_CACHE = {}


def kernel(hidden_states, Wq, bq, Wk, bk, Wv, bv):
    if "nc" not in _NC_CACHE:
        _NC_CACHE["nc"] = build_program(split_waits=True)
    nc = _NC_CACHE["nc"]
    in_maps = make_in_maps(hidden_states, Wq, bq, Wk, bk, Wv, bv)
    res = run_bass_kernel_spmd(nc, in_maps, core_ids=list(range(NCORE)))
    return assemble_out([res.results[r]["out"] for r in range(NCORE)])


def assemble_out(per_core):
    """8 x [S, C] -> [B, S, E]."""
    full = np.stack(
        [
            np.concatenate(per_core[b * (NCORE // B) : (b + 1) * (NCORE // B)], axis=1)
            for b in range(B)
        ]
    )
    return np.ascontiguousarray(full).astype(np.float32)
